# revision 23
# baseline (speedup 1.0000x reference)
"""Trainium2 Bass kernel for nn_CausalWanSelfAttention_45904610460041.

Strategy (8 NeuronCores, full I/O), v3:
  Launch 1 (column-sharded): each core computes x @ [wq|wk|wv]-cols for its
    576 output columns (1.5 heads' worth of q, k and v) over all 1560 rows
    in bf16 (fp32 PSUM).  RMS-ssq, rope, rms scaling all happen on the host
    between launches (index/elementwise glue only), so the device work is
    pure matmul + cast.
  Host glue: sum-of-squares -> rms scales, rope applied to q/k (g folded),
    KV-cache roll/update/window indexing (numpy, index-only), effective
    K/V assembly and launch-2 layouts.
  Launch 2 (2 query-blocks x 4 head-groups grid): core (qg, hg) takes
    780 queries x 3 heads x all 4680 keys.  Logits^T per 128-key chunk
    (keys on PSUM partitions, two chunks per 2-bank PSUM tile, N=390),
    exp on ScalarE (scale=1/sqrt(d), bias=-1 folded in; the shift cancels
    in softmax), P.V accumulated per key chunk into O^T, denominators via
    VectorE bf16 pair-tile folding plus a final ones-matmul partition
    reduce, then per-head o-projection partials out_h^T = wo_h^T @ O_h^T.
    Host divides by the denominators, sums partials over heads/groups and
    adds bo.
"""

import os
import sys

for _p in ("/opt/trn_rl_repo",):
    if os.path.isdir(_p) and _p not in sys.path:
        sys.path.insert(0, _p)

import numpy as np
import ml_dtypes

import concourse.bass as bass
import concourse.tile as tile
from concourse import bacc
from concourse import mybir
from concourse import bass_utils
from concourse.alu_op_type import AluOpType

BF16 = ml_dtypes.bfloat16
AF = mybir.ActivationFunctionType

# ---------------------------------------------------------------------------
# Problem constants (fixed by the input specs).
S = 1560          # query/new-token sequence length
DIM = 1536
NH = 12
HD = 128
CACHE = 4680      # kv cache length == effective attention keys here
NCORES = 8
EPS = 1e-6
LOCAL_ATTN_SIZE = 3
SINK_SIZE = 1
MAX_ATTN = 32760 if LOCAL_ATTN_SIZE == -1 else LOCAL_ATTN_SIZE * S

NKC = (CACHE + 127) // 128      # 37 key chunks
TAIL = CACHE - (NKC - 1) * 128  # 72 keys in the tail chunk
NPAIR = (NKC + 1) // 2          # 19 chunk pairs (pair 18 = tail alone)

# Launch-1 grid: 13 row chunks of 120 rows; per-core 576 columns.
L1_RC = 120
L1_NRC = S // L1_RC             # 13
L1_COLS = 576                   # per-core columns (q 192 | k 192 | v 192)
CPC = DIM // NCORES             # 192 q (or k, v) columns per core

# Launch-2 grid: 2 query blocks x 4 head groups.
QB = S // 2                     # 780 queries per core
HPC = 3                         # heads per core
QN = 390                        # matmul N (two halves of 780)
ATT_SCALE = 1.0 / float(np.sqrt(HD))
EXP_SHIFT = 1.0                 # exp(s*L - EXP_SHIFT); cancels in softmax
# Schraudolph bit-trick exp -> bf16 bit pattern via round(L*A16 + B16) as
# uint16 (fp32->int conversion rounds to nearest on DVE, hw-verified);
# -7.33 de-biases the piecewise-linear 2^frac approximation.
SCH_A16 = ATT_SCALE * 128.0 / np.log(2.0)
SCH_B16 = 16256.0 - EXP_SHIFT * 128.0 / np.log(2.0) - 7.33
SCH_PAIRS = frozenset(range(3, NPAIR - 1, 4))   # pairs exp'd on VectorE

_CACHED = {}
LAST_RUNS = []  # BassKernelResults of the most recent kernel() call


# ---------------------------------------------------------------------------
def _build_launch1():
    nc = bacc.Bacc("TRN2", target_bir_lowering=False, debug=False,
                   num_devices=NCORES, num_swdge_queues=4)
    f32, bf = mybir.dt.float32, mybir.dt.bfloat16

    xt_d = nc.dram_tensor("xt", [128, 12, S], bf, kind="ExternalInput")
    wp_d = nc.dram_tensor("wp", [128, 12, L1_COLS], bf, kind="ExternalInput")
    out_d = nc.dram_tensor("qkv", [L1_NRC, L1_RC, L1_COLS], bf,
                           kind="ExternalOutput")

    with tile.TileContext(nc) as tc:
        with (
            tc.tile_pool(name="consts", bufs=1) as consts,
            tc.tile_pool(name="ps", bufs=4, space="PSUM") as psp,
            tc.tile_pool(name="outs", bufs=3) as outsp,
        ):
            xt = consts.tile([128, 12, S], bf)
            wt = consts.tile([128, 12, L1_COLS], bf)
            # weights first (needed by every row chunk); descriptor
            # generation stays off ScalarE (it does the psum casts)
            nc.gpsimd.dma_start(wt[:, 0:6], wp_d.ap()[:, 0:6])
            nc.sync.dma_start(xt[:, 0:2], xt_d.ap()[:, 0:2])
            nc.gpsimd.dma_start(wt[:, 6:12], wp_d.ap()[:, 6:12])
            nc.sync.dma_start(xt[:, 2:4], xt_d.ap()[:, 2:4])
            nc.gpsimd.dma_start(xt[:, 4:8], xt_d.ap()[:, 4:8])
            nc.sync.dma_start(xt[:, 8:12], xt_d.ap()[:, 8:12])

            # PE warmup: trip the clock gate before the main stream
            wsrc = consts.tile([128, 512], bf, name="wsrc")
            nc.vector.memset(wsrc[:], 0.0)
            for wu in range(8):
                wp_ = psp.tile([128, 2, 512], f32, tag="ps", name="wp_")
                nc.tensor.matmul(wp_[:, 0, :], wsrc[:, :128], wsrc[:],
                                 start=True, stop=True)

            for m in range(L1_NRC):
                r0 = m * L1_RC
                ps = psp.tile([128, 2, 512], f32, tag="ps", name="ps")
                for ns in range(2):
                    for kc in range(12):
                        nc.tensor.matmul(
                            ps[:L1_RC, ns, 0:288],
                            xt[:, kc, r0:r0 + L1_RC],
                            wt[:, kc, ns * 288:(ns + 1) * 288],
                            start=(kc == 0), stop=(kc == 11))
                ot = outsp.tile([128, L1_COLS], bf, tag="ot", name="ot")
                # split the cast across engines so the PSUM tile frees fast
                nc.scalar.activation(out=ot[:L1_RC, 0:288],
                                     in_=ps[:L1_RC, 0, 0:288],
                                     func=AF.Copy)
                nc.vector.tensor_copy(ot[:L1_RC, 288:576],
                                      ps[:L1_RC, 1, 0:288])
                (nc.sync, nc.gpsimd)[m % 2].dma_start(
                    out_d.ap()[m], ot[:L1_RC, :])

    nc.finalize()
    return nc


# ---------------------------------------------------------------------------
def _build_launch2():
    nc = bacc.Bacc("TRN2", target_bir_lowering=False, debug=False,
                   num_devices=NCORES, num_swdge_queues=4)
    f32, bf = mybir.dt.float32, mybir.dt.bfloat16
    u16 = mybir.dt.uint16

    qt_d = nc.dram_tensor("qt", [128, HPC, QB], bf, kind="ExternalInput")
    kt_d = nc.dram_tensor("kt", [HPC, 128, CACHE], bf, kind="ExternalInput")
    vt_d = nc.dram_tensor("vt", [HPC, 128, NKC, HD], bf,
                          kind="ExternalInput")
    w2_d = nc.dram_tensor("w2", [128, HPC, 12, 128], bf, kind="ExternalInput")
    out_d = nc.dram_tensor("outp", [HPC, 12, 128, QB], bf,
                           kind="ExternalOutput")
    ds_d = nc.dram_tensor("dsum", [HPC, 2, QN], f32, kind="ExternalOutput")

    with tile.TileContext(nc) as tc:
        with (
            tc.tile_pool(name="consts", bufs=1) as consts,
            tc.tile_pool(name="kv", bufs=2) as kvp,
            tc.tile_pool(name="p", bufs=3) as pp,
            tc.tile_pool(name="acc", bufs=2) as accp,
            tc.tile_pool(name="lp", bufs=3, space="PSUM") as lpp,
            tc.tile_pool(name="ops", bufs=1, space="PSUM") as opsp,
            tc.tile_pool(name="dr", bufs=2, space="PSUM") as drp,
            tc.tile_pool(name="o3", bufs=1) as o3p,
            tc.tile_pool(name="outs", bufs=4) as outsp,
        ):
            qt = consts.tile([128, HPC, QB], bf)
            nc.gpsimd.dma_start(qt[:, 0], qt_d.ap()[:, 0])
            w2 = consts.tile([128, HPC, 12, 128], bf)
            ones_bf = consts.tile([128, 1], bf)
            nc.vector.memset(ones_bf[:], 1.0)
            ebias = consts.tile([128, 1], f32)
            nc.vector.memset(ebias[:], -EXP_SHIFT)

            # PE warmup
            wsrc = consts.tile([128, 512], bf, name="wsrc")
            nc.vector.memset(wsrc[:], 0.0)
            for wu in range(8):
                wp_ = lpp.tile([128, 2, 512], f32, tag="lp", name="wp_")
                nc.tensor.matmul(wp_[:, 0, :], wsrc[:, :128], wsrc[:],
                                 start=True, stop=True)

            o3 = o3p.tile([128, HPC, QB], bf)   # unnormalized O^T per head

            DEPTH = 2       # QK pairs emitted ahead of the matching PV
            NQUAD = (NPAIR + 1) // 2   # 10 quads (quad 9 = tail pair alone)
            pending_dred = []

            oproj_ready = []
            oproj_n = [0]

            def emit_oproj_half(tag="po"):
                if not oproj_ready:
                    return
                tp, m, qg = oproj_ready.pop(0)
                k = oproj_n[0]
                oproj_n[0] += 1
                po = opsp.tile([128, QN], f32, tag=tag, name="po")
                nc.tensor.matmul(
                    po[:], w2[:, tp, m, :],
                    o3[:, tp, qg * QN:(qg + 1) * QN],
                    start=True, stop=True)
                ob = outsp.tile([128, QN], bf, tag="ob", name="ob")
                if k % 2 == 0:
                    nc.scalar.activation(out=ob[:], in_=po[:],
                                         func=AF.Copy)
                else:
                    nc.vector.tensor_copy(ob[:], po[:])
                (nc.sync, nc.gpsimd)[k % 2].dma_start(
                    out_d.ap()[tp][m][:, qg * QN:(qg + 1) * QN], ob[:])

            for t in range(HPC):
                kt = kvp.tile([128, CACHE], bf, tag="kt", name="ktile")
                vt = kvp.tile([128, NKC, HD], bf, tag="vt", name="vtile")
                if t == 0:
                    # criticality-ordered: small first-needed pieces land
                    # fast; bulk descriptor generation stays off ScalarE
                    nc.sync.dma_start(vt[:, 0:2], vt_d.ap()[t][:, 0:2])
                    nc.sync.dma_start(kt[:, 0:585], kt_d.ap()[t][:, 0:585])
                    nc.gpsimd.dma_start(vt[:, 2:10], vt_d.ap()[t][:, 2:10])
                    nc.sync.dma_start(kt[:, 585:2340],
                                      kt_d.ap()[t][:, 585:2340])
                    nc.gpsimd.dma_start(vt[:, 10:37], vt_d.ap()[t][:, 10:37])
                    nc.sync.dma_start(kt[:, 2340:CACHE],
                                      kt_d.ap()[t][:, 2340:CACHE])
                    nc.gpsimd.dma_start(qt[:, 1:3], qt_d.ap()[:, 1:3])
                    nc.gpsimd.dma_start(w2[:], w2_d.ap())
                else:
                    nc.sync.dma_start(kt[:], kt_d.ap()[t])
                    nc.gpsimd.dma_start(vt[:], vt_d.ap()[t])

                for qg in range(2):
                    q0 = qg * QN
                    opsum = opsp.tile([128, QN], f32, tag="opsum",
                                      name="opsum")
                    dacc4 = accp.tile([128, 4, QN], bf, tag="dacc",
                                      name="dacc")
                    lps = {}

                    def emit_qk(pj):
                        lp = lpp.tile([128, 2, 512], f32, tag="lp",
                                      name="lp")
                        lps[pj] = lp
                        if pj < NPAIR - 1:
                            for i in range(2):
                                j = 2 * pj + i
                                nc.tensor.matmul(
                                    lp[:, i, 0:QN],
                                    kt[:, j * 128:(j + 1) * 128],
                                    qt[:, t, q0:q0 + QN],
                                    start=True, stop=True)
                        else:
                            j = 2 * pj
                            nc.tensor.matmul(
                                lp[:TAIL, 0, 0:QN],
                                kt[:, j * 128:j * 128 + TAIL],
                                qt[:, t, q0:q0 + QN],
                                start=True, stop=True)

                    for pj in range(min(DEPTH + 1, NPAIR)):
                        emit_qk(pj)

                    for du in range(NQUAD):
                        ptq = pp.tile([128, 4, QN], bf, tag="pt", name="pt")
                        for half in range(2):
                            pj = 2 * du + half
                            if pj >= NPAIR:
                                continue
                            lp = lps.pop(pj)
                            last = pj == NPAIR - 1
                            sl = slice(2 * half, 2 * half + 2)
                            if not last:
                                if pj in SCH_PAIRS:
                                    nc.vector.tensor_scalar(
                                        out=ptq[:, sl, :].bitcast(u16),
                                        in0=lp[:, :, 0:QN],
                                        scalar1=float(SCH_A16),
                                        scalar2=float(SCH_B16),
                                        op0=AluOpType.mult,
                                        op1=AluOpType.add)
                                else:
                                    nc.scalar.activation(
                                        out=ptq[:, sl, :],
                                        in_=lp[:, :, 0:QN],
                                        func=AF.Exp, scale=ATT_SCALE,
                                        bias=ebias[:])
                            else:
                                nc.scalar.activation(
                                    out=ptq[:TAIL, 2 * half, :],
                                    in_=lp[:TAIL, 0, 0:QN],
                                    func=AF.Exp, scale=ATT_SCALE,
                                    bias=ebias[:TAIL])
                            if not last:
                                for i in range(2):
                                    j = 2 * pj + i
                                    nc.tensor.matmul(
                                        opsum[:], vt[:, j, :],
                                        ptq[:, 2 * half + i, :],
                                        start=(j == 0), stop=False)
                            else:
                                j = 2 * pj
                                nc.tensor.matmul(
                                    opsum[:], vt[0:TAIL, j, :],
                                    ptq[:TAIL, 2 * half, :],
                                    start=False, stop=True)
                            if pj + DEPTH + 1 < NPAIR:
                                emit_qk(pj + DEPTH + 1)
                        # one denominator fold per quad (FD=1560, bf16 2x)
                        if du == 0:
                            nc.vector.tensor_copy(dacc4[:], ptq[:])
                        elif 2 * du + 1 < NPAIR:
                            nc.vector.tensor_tensor(
                                dacc4[:], dacc4[:], ptq[:], AluOpType.add)
                        else:
                            nc.vector.tensor_tensor(
                                dacc4[:TAIL, 0, :], dacc4[:TAIL, 0, :],
                                ptq[:TAIL, 0, :], AluOpType.add)
                        # deferred boundary work from the previous block,
                        # emitted once the PE is warm again
                        if du == 1 and pending_dred:
                            dp, tp, qp = pending_dred.pop(0)
                            dred = lpp.tile([128, 2, 512], f32, tag="lp",
                                            name="dred")
                            nc.tensor.matmul(dred[0:1, 0, 0:QN],
                                             ones_bf[:], dp[:, 0, :],
                                             start=True, stop=True)
                            dsb = outsp.tile([1, QN], f32, tag="dsb",
                                             name="dsb")
                            nc.vector.tensor_copy(dsb[:],
                                                  dred[0:1, 0, 0:QN])
                            nc.gpsimd.dma_start(ds_d.ap()[tp][qp], dsb[:])
                        # interleave ready o-projection halves
                        if du >= 1:
                            emit_oproj_half()
                            if du >= 3:
                                emit_oproj_half()

                    # combine fold slots (VectorE); the partition-reduce
                    # matmul is deferred into the next block
                    for sl2 in (1, 2, 3):
                        nc.vector.tensor_tensor(
                            dacc4[:, 0, :], dacc4[:, 0, :],
                            dacc4[:, sl2, :], AluOpType.add)
                    pending_dred.append((dacc4, t, qg))
                    # stage O^T bf16 for the o-projection
                    nc.vector.tensor_copy(o3[:, t, q0:q0 + QN], opsum[:])
                    oproj_ready.extend((t, m, qg) for m in range(12))

            # flush remaining denominators and the last head's o-projection
            for dp, tp, qp in pending_dred:
                dred = lpp.tile([128, 2, 512], f32, tag="lp", name="dred")
                nc.tensor.matmul(dred[0:1, 0, 0:QN], ones_bf[:],
                                 dp[:, 0, :], start=True, stop=True)
                dsb = outsp.tile([1, QN], f32, tag="dsb", name="dsb")
                nc.vector.tensor_copy(dsb[:], dred[0:1, 0, 0:QN])
                nc.gpsimd.dma_start(ds_d.ap()[tp][qp], dsb[:])
            k = 0
            while oproj_ready:
                emit_oproj_half(tag=("po", "opsum")[k % 2])
                k += 1

    nc.finalize()
    return nc


# ---------------------------------------------------------------------------
def _cache_plan(current_start, global_end_index, local_end_index, s, kv_size,
                frame_seqlen):
    """Numpy re-implementation of the reference's cache roll/update/window
    logic, tracking only *indices*: returns (old_cache_rows, new_rows) such
    that the attended key set == cache[old_cache_rows] ++ new[new_rows]."""
    current_end = current_start + s
    sink_tokens = SINK_SIZE * frame_seqlen

    kind = np.zeros(kv_size, dtype=np.int64)
    idx = np.arange(kv_size, dtype=np.int64)

    if (LOCAL_ATTN_SIZE != -1 and current_end > global_end_index
            and s + local_end_index > kv_size):
        num_evicted = s + local_end_index - kv_size
        num_rolled = local_end_index - num_evicted - sink_tokens
        src0 = sink_tokens + num_evicted
        kind[sink_tokens:sink_tokens + num_rolled] = \
            kind[src0:src0 + num_rolled]
        idx[sink_tokens:sink_tokens + num_rolled] = \
            idx[src0:src0 + num_rolled]
        new_local_end = (local_end_index + current_end - global_end_index
                         - num_evicted)
    else:
        new_local_end = local_end_index + current_end - global_end_index
    local_start = new_local_end - s
    is_recompute = (current_end <= global_end_index) and (current_start > 0)
    write_start = max(local_start, sink_tokens) if is_recompute \
        else local_start
    off = max(0, write_start - local_start)
    wl = max(0, new_local_end - write_start)
    if wl > 0:
        kind[write_start:new_local_end] = 1
        idx[write_start:new_local_end] = off + np.arange(wl)

    if sink_tokens > 0:
        budget = MAX_ATTN - sink_tokens
        if budget > 0:
            lo = max(sink_tokens, new_local_end - budget)
            sel = np.concatenate([np.arange(sink_tokens),
                                  np.arange(lo, new_local_end)])
        else:
            sel = np.arange(sink_tokens)
    else:
        ws = max(0, new_local_end - MAX_ATTN)
        sel = np.arange(ws, new_local_end)

    k_kind, k_idx = kind[sel], idx[sel]
    old_rows = k_idx[k_kind == 0]
    new_rows = k_idx[k_kind == 1]
    return old_rows, new_rows


def _rope_tables(freqs_real, freqs_imag, f, h, w, start_frame):
    """(S, HD) cos table and sign-folded sin table for one head."""
    c = HD // 2  # 64
    c0 = c - 2 * (c // 3)
    c1 = c // 3
    fr = np.asarray(freqs_real, np.float32)
    fi = np.asarray(freqs_imag, np.float32)
    s = f * h * w
    assert s == S
    fidx = np.arange(s) // (h * w)
    hidx = (np.arange(s) // w) % h
    widx = np.arange(s) % w
    fr_pos = np.concatenate([
        fr[start_frame + fidx][:, :c0],
        fr[hidx][:, c0:c0 + c1],
        fr[widx][:, c0 + c1:c0 + 2 * c1],
    ], axis=1)  # (S, 64)
    fi_pos = np.concatenate([
        fi[start_frame + fidx][:, :c0],
        fi[hidx][:, c0:c0 + c1],
        fi[widx][:, c0 + c1:c0 + 2 * c1],
    ], axis=1)
    C1 = np.repeat(fr_pos, 2, axis=1)              # (S, 128) cos
    Sg = np.empty((s, HD), np.float32)
    Sg[:, 0::2] = -fi_pos                          # y_even = xe*c - xo*si
    Sg[:, 1::2] = fi_pos                           # y_odd  = xo*c + xe*si
    return C1, Sg


def _rope_apply(x, C, Sx, g):
    """x: (S, DIM) float32; returns rope(x*g) per head with g folded."""
    gx = x * np.asarray(g, np.float32)[None, :]
    xs = gx.reshape(S, NH, HD // 2, 2)
    sw = xs[..., ::-1].reshape(S, NH, HD)          # swapped pairs
    xr = gx.reshape(S, NH, HD)
    return (xr * C[:, None, :] + sw * Sx[:, None, :]).reshape(S, DIM)


# ---------------------------------------------------------------------------
def kernel(x, cache_k, cache_v, freqs_real, freqs_imag,
           wq, bq, wk, bk, wv, bv, wo, bo, gq, gk,
           f_frames, height, width, current_start, global_end_index,
           local_end_index):
    global LAST_RUNS
    LAST_RUNS = []

    x = np.asarray(x, np.float32)
    cache_k = np.asarray(cache_k, np.float32)
    cache_v = np.asarray(cache_v, np.float32)
    wq = np.asarray(wq, np.float32)
    wk = np.asarray(wk, np.float32)
    wv = np.asarray(wv, np.float32)
    wo = np.asarray(wo, np.float32)
    bo = np.asarray(bo, np.float32)
    f = int(f_frames)
    h = int(height)
    w = int(width)
    current_start = int(current_start)
    global_end_index = int(global_end_index)
    local_end_index = int(local_end_index)

    assert x.shape == (1, S, DIM)
    for b in (bq, bk, bv):
        assert not np.any(np.asarray(b)), "nonzero qkv bias unsupported"

    frame_seqlen = h * w
    start_frame = current_start // frame_seqlen

    # ---- launch 1: q/k/v projections (column-sharded, bf16) ----
    xT = np.ascontiguousarray(x[0].T)                       # (1536, 1560)
    xtp = np.ascontiguousarray(
        xT.reshape(12, 128, S).transpose(1, 0, 2)).astype(BF16)

    nc1 = _CACHED.get("l1")
    if nc1 is None:
        nc1 = _CACHED["l1"] = _build_launch1()

    in_maps1 = []
    for c in range(NCORES):
        cs = slice(c * CPC, (c + 1) * CPC)
        W_slice = np.concatenate([wq[:, cs], wk[:, cs], wv[:, cs]], axis=1)
        wp = np.ascontiguousarray(
            W_slice.reshape(12, 128, L1_COLS).transpose(1, 0, 2)).astype(BF16)
        in_maps1.append({"xt": xtp, "wp": wp})
    res1 = bass_utils.run_bass_kernel_spmd(nc1, in_maps1,
                                           core_ids=list(range(NCORES)))
    LAST_RUNS.append(res1)

    Q = np.empty((S, DIM), np.float32)
    K = np.empty((S, DIM), np.float32)
    V = np.empty((S, DIM), np.float32)
    for c in range(NCORES):
        cs = slice(c * CPC, (c + 1) * CPC)
        blk = res1.results[c]["qkv"].reshape(S, L1_COLS).astype(np.float32)
        Q[:, cs] = blk[:, 0:CPC]
        K[:, cs] = blk[:, CPC:2 * CPC]
        V[:, cs] = blk[:, 2 * CPC:3 * CPC]

    # ---- host glue: rms + rope + cache assembly ----
    rs_q = 1.0 / np.sqrt(np.mean(Q * Q, axis=1, keepdims=True) + EPS)
    rs_k = 1.0 / np.sqrt(np.mean(K * K, axis=1, keepdims=True) + EPS)
    C1, Sg = _rope_tables(freqs_real, freqs_imag, f, h, w, start_frame)
    Qr = _rope_apply(Q, C1, Sg, gq) * rs_q
    Kr = _rope_apply(K, C1, Sg, gk) * rs_k

    old_rows, new_rows = _cache_plan(current_start, global_end_index,
                                     local_end_index, S, cache_k.shape[1],
                                     frame_seqlen)
    n_old = len(old_rows)
    assert n_old + len(new_rows) == CACHE, "unexpected key count"

    K_eff = np.empty((CACHE, DIM), np.float32)
    V_eff = np.empty((CACHE, DIM), np.float32)
    K_eff[:n_old] = cache_k[0, old_rows].reshape(n_old, DIM)
    K_eff[n_old:] = Kr[new_rows]
    V_eff[:n_old] = cache_v[0, old_rows].reshape(n_old, DIM)
    V_eff[n_old:] = V[new_rows]

    Q8 = Qr.astype(BF16)                                    # (S, DIM)
    K8 = K_eff.astype(BF16)
    V8 = V_eff.astype(BF16)

    # launch-2 layouts
    V_pad = np.zeros((NKC * 128, DIM), BF16)
    V_pad[:CACHE] = V8
    # vt[h, p, j, d] = V[j*128 + p, h*128 + d]
    vt_all = np.ascontiguousarray(
        V_pad.reshape(NKC, 128, NH, HD).transpose(2, 1, 0, 3))
    # kt[h, d, key]
    kt_all = np.ascontiguousarray(
        K8.T.reshape(NH, HD, CACHE))
    # qt[h, d, row]
    qt_all = np.ascontiguousarray(Q8.T.reshape(NH, HD, S))
    wo_bf = wo.astype(BF16)

    nc2 = _CACHED.get("l2")
    if nc2 is None:
        nc2 = _CACHED["l2"] = _build_launch2()

    in_maps2 = []
    for c in range(NCORES):
        qg, hg = divmod(c, 4)
        hs = slice(hg * HPC, (hg + 1) * HPC)
        w2 = np.ascontiguousarray(
            wo_bf[hg * HPC * 128:(hg + 1) * HPC * 128].reshape(
                HPC, 128, 12, 128).transpose(1, 0, 2, 3))
        in_maps2.append({
            "qt": np.ascontiguousarray(
                qt_all[hs, :, qg * QB:(qg + 1) * QB].transpose(1, 0, 2)),
            "kt": np.ascontiguousarray(kt_all[hs]),
            "vt": np.ascontiguousarray(vt_all[hs]),
            "w2": w2,
        })
    res2 = bass_utils.run_bass_kernel_spmd(nc2, in_maps2,
                                           core_ids=list(range(NCORES)))
    LAST_RUNS.append(res2)

    # ---- host: normalize by denominators, reduce heads, add bo ----
    out = np.zeros((S, DIM), np.float32)
    for c in range(NCORES):
        qg, hg = divmod(c, 4)
        o_part = res2.results[c]["outp"].astype(np.float32)  # [3,12,128,QB]
        dsum = res2.results[c]["dsum"].reshape(HPC, QB)      # [3, QB]
        rows = slice(qg * QB, (qg + 1) * QB)
        acc = (o_part / dsum[:, None, None, :]).sum(axis=0)  # [12,128,QB]
        out[rows] += acc.reshape(DIM, QB).T
    out += bo[None, :]
    return out.reshape(1, S, DIM)


# revision 24
# speedup vs baseline: 1.1989x; 1.1989x over previous
"""Trainium2 Bass kernel for nn_CausalWanSelfAttention_45904610460041.

Strategy (8 NeuronCores, full I/O), v3:
  Launch 1 (column-sharded): each core computes x @ [wq|wk|wv]-cols for its
    576 output columns (1.5 heads' worth of q, k and v) over all 1560 rows
    in bf16 (fp32 PSUM).  RMS-ssq, rope, rms scaling all happen on the host
    between launches (index/elementwise glue only), so the device work is
    pure matmul + cast.
  Host glue: sum-of-squares -> rms scales, rope applied to q/k (g folded),
    KV-cache roll/update/window indexing (numpy, index-only), effective
    K/V assembly and launch-2 layouts.
  Launch 2 (2 query-blocks x 4 head-groups grid): core (qg, hg) takes
    780 queries x 3 heads x all 4680 keys.  Logits^T per 128-key chunk
    (keys on PSUM partitions, two chunks per 2-bank PSUM tile, N=390),
    exp on ScalarE (scale=1/sqrt(d), bias=-1 folded in; the shift cancels
    in softmax), P.V accumulated per key chunk into O^T, denominators via
    VectorE bf16 pair-tile folding plus a final ones-matmul partition
    reduce, then per-head o-projection partials out_h^T = wo_h^T @ O_h^T.
    Host divides by the denominators, sums partials over heads/groups and
    adds bo.
"""

import os
import sys

for _p in ("/opt/trn_rl_repo",):
    if os.path.isdir(_p) and _p not in sys.path:
        sys.path.insert(0, _p)

import numpy as np
import ml_dtypes

import concourse.bass as bass
import concourse.tile as tile
from concourse import bacc
from concourse import mybir
from concourse import bass_utils
from concourse.alu_op_type import AluOpType

BF16 = ml_dtypes.bfloat16
AF = mybir.ActivationFunctionType

# ---------------------------------------------------------------------------
# Problem constants (fixed by the input specs).
S = 1560          # query/new-token sequence length
DIM = 1536
NH = 12
HD = 128
CACHE = 4680      # kv cache length == effective attention keys here
NCORES = 8
EPS = 1e-6
LOCAL_ATTN_SIZE = 3
SINK_SIZE = 1
MAX_ATTN = 32760 if LOCAL_ATTN_SIZE == -1 else LOCAL_ATTN_SIZE * S

NKC = (CACHE + 127) // 128      # 37 key chunks
TAIL = CACHE - (NKC - 1) * 128  # 72 keys in the tail chunk
NPAIR = (NKC + 1) // 2          # 19 chunk pairs (pair 18 = tail alone)

# Launch-1 grid: 13 row chunks of 120 rows; per-core 576 columns.
L1_RC = 120
L1_NRC = S // L1_RC             # 13
L1_COLS = 576                   # per-core columns (q 192 | k 192 | v 192)
CPC = DIM // NCORES             # 192 q (or k, v) columns per core

# Launch-2 grid: 2 query blocks x 4 head groups.
QB = S // 2                     # 780 queries per core
HPC = 3                         # heads per core
QN = 390                        # matmul N (two halves of 780)
ATT_SCALE = 1.0 / float(np.sqrt(HD))
EXP_SHIFT = 1.0                 # exp(s*L - EXP_SHIFT); cancels in softmax
# Schraudolph bit-trick exp -> bf16 bit pattern via round(L*A16 + B16) as
# uint16 (fp32->int conversion rounds to nearest on DVE, hw-verified);
# -7.33 de-biases the piecewise-linear 2^frac approximation.
SCH_A16 = ATT_SCALE * 128.0 / np.log(2.0)
SCH_B16 = 16256.0 - EXP_SHIFT * 128.0 / np.log(2.0) - 7.33
SCH_PAIRS = frozenset(range(3, NPAIR - 1, 4))   # pairs exp'd on VectorE

_CACHED = {}
LAST_RUNS = []  # BassKernelResults of the most recent kernel() call


# ---------------------------------------------------------------------------
def _build_launch1():
    nc = bacc.Bacc("TRN2", target_bir_lowering=False, debug=False,
                   num_devices=NCORES, num_swdge_queues=4)
    f32, bf = mybir.dt.float32, mybir.dt.bfloat16

    xt_d = nc.dram_tensor("xt", [128, 12, S], bf, kind="ExternalInput")
    wp_d = nc.dram_tensor("wp", [128, 12, L1_COLS], bf, kind="ExternalInput")
    out_d = nc.dram_tensor("qkv", [L1_NRC, L1_RC, L1_COLS], bf,
                           kind="ExternalOutput")

    with tile.TileContext(nc) as tc:
        with (
            tc.tile_pool(name="consts", bufs=1) as consts,
            tc.tile_pool(name="ps", bufs=4, space="PSUM") as psp,
            tc.tile_pool(name="outs", bufs=3) as outsp,
        ):
            xt = consts.tile([128, 12, S], bf)
            wt = consts.tile([128, 12, L1_COLS], bf)
            # weights first (needed by every row chunk), xt spread across
            # queues in contraction order
            nc.scalar.dma_start(wt[:, 0:6], wp_d.ap()[:, 0:6])
            nc.gpsimd.dma_start(wt[:, 6:12], wp_d.ap()[:, 6:12])
            nc.sync.dma_start(xt[:, 0:2], xt_d.ap()[:, 0:2])
            nc.scalar.dma_start(xt[:, 2:4], xt_d.ap()[:, 2:4])
            nc.gpsimd.dma_start(xt[:, 4:6], xt_d.ap()[:, 4:6])
            nc.sync.dma_start(xt[:, 6:8], xt_d.ap()[:, 6:8])
            nc.scalar.dma_start(xt[:, 8:10], xt_d.ap()[:, 8:10])
            nc.gpsimd.dma_start(xt[:, 10:12], xt_d.ap()[:, 10:12])

            # PE warmup: trip the clock gate before the main stream
            wsrc = consts.tile([128, 512], bf, name="wsrc")
            nc.vector.memset(wsrc[:], 0.0)
            for wu in range(8):
                wp_ = psp.tile([128, 2, 512], f32, tag="ps", name="wp_")
                nc.tensor.matmul(wp_[:, 0, :], wsrc[:, :128], wsrc[:],
                                 start=True, stop=True)

            for m in range(L1_NRC):
                r0 = m * L1_RC
                ps = psp.tile([128, 2, 512], f32, tag="ps", name="ps")
                for ns in range(2):
                    for kc in range(12):
                        nc.tensor.matmul(
                            ps[:L1_RC, ns, 0:288],
                            xt[:, kc, r0:r0 + L1_RC],
                            wt[:, kc, ns * 288:(ns + 1) * 288],
                            start=(kc == 0), stop=(kc == 11))
                ot = outsp.tile([128, L1_COLS], bf, tag="ot", name="ot")
                # split the cast across engines so the PSUM tile frees fast
                nc.scalar.activation(out=ot[:L1_RC, 0:288],
                                     in_=ps[:L1_RC, 0, 0:288],
                                     func=AF.Copy)
                nc.vector.tensor_copy(ot[:L1_RC, 288:576],
                                      ps[:L1_RC, 1, 0:288])
                (nc.sync, nc.scalar, nc.gpsimd)[m % 3].dma_start(
                    out_d.ap()[m], ot[:L1_RC, :])

    nc.finalize()
    return nc


# ---------------------------------------------------------------------------
def _build_launch2():
    nc = bacc.Bacc("TRN2", target_bir_lowering=False, debug=False,
                   num_devices=NCORES, num_swdge_queues=4)
    f32, bf = mybir.dt.float32, mybir.dt.bfloat16
    u16 = mybir.dt.uint16

    qt_d = nc.dram_tensor("qt", [128, HPC, QB], bf, kind="ExternalInput")
    kt_d = nc.dram_tensor("kt", [HPC, 128, CACHE], bf, kind="ExternalInput")
    vt_d = nc.dram_tensor("vt", [HPC, 128, NKC, HD], bf,
                          kind="ExternalInput")
    w2_d = nc.dram_tensor("w2", [128, HPC, 12, 128], bf, kind="ExternalInput")
    out_d = nc.dram_tensor("outp", [HPC, 12, 128, QB], bf,
                           kind="ExternalOutput")
    ds_d = nc.dram_tensor("dsum", [HPC, 2, QN], f32, kind="ExternalOutput")

    with tile.TileContext(nc) as tc:
        with (
            tc.tile_pool(name="consts", bufs=1) as consts,
            tc.tile_pool(name="kv", bufs=2) as kvp,
            tc.tile_pool(name="p", bufs=3) as pp,
            tc.tile_pool(name="acc", bufs=2) as accp,
            tc.tile_pool(name="lp", bufs=3, space="PSUM") as lpp,
            tc.tile_pool(name="ops", bufs=1, space="PSUM") as opsp,
            tc.tile_pool(name="dr", bufs=2, space="PSUM") as drp,
            tc.tile_pool(name="o3", bufs=1) as o3p,
            tc.tile_pool(name="outs", bufs=4) as outsp,
        ):
            qt = consts.tile([128, HPC, QB], bf)
            nc.scalar.dma_start(qt[:, 0], qt_d.ap()[:, 0])
            nc.gpsimd.dma_start(qt[:, 1:3], qt_d.ap()[:, 1:3])
            w2 = consts.tile([128, HPC, 12, 128], bf)
            nc.gpsimd.dma_start(w2[:], w2_d.ap())
            ones_bf = consts.tile([128, 1], bf)
            nc.vector.memset(ones_bf[:], 1.0)
            ebias = consts.tile([128, 1], f32)
            nc.vector.memset(ebias[:], -EXP_SHIFT)

            # PE warmup
            wsrc = consts.tile([128, 512], bf, name="wsrc")
            nc.vector.memset(wsrc[:], 0.0)
            for wu in range(8):
                wp_ = lpp.tile([128, 2, 512], f32, tag="lp", name="wp_")
                nc.tensor.matmul(wp_[:, 0, :], wsrc[:, :128], wsrc[:],
                                 start=True, stop=True)

            o3 = o3p.tile([128, HPC, QB], bf)   # unnormalized O^T per head

            DEPTH = 2       # QK pairs emitted ahead of the matching PV
            NQUAD = (NPAIR + 1) // 2   # 10 quads (quad 9 = tail pair alone)
            pending_dred = []

            oproj_ready = []
            oproj_n = [0]

            def emit_oproj_half(tag="po"):
                if not oproj_ready:
                    return
                tp, m, qg = oproj_ready.pop(0)
                k = oproj_n[0]
                oproj_n[0] += 1
                po = opsp.tile([128, QN], f32, tag=tag, name="po")
                nc.tensor.matmul(
                    po[:], w2[:, tp, m, :],
                    o3[:, tp, qg * QN:(qg + 1) * QN],
                    start=True, stop=True)
                ob = outsp.tile([128, QN], bf, tag="ob", name="ob")
                if k % 2 == 0:
                    nc.scalar.activation(out=ob[:], in_=po[:],
                                         func=AF.Copy)
                else:
                    nc.vector.tensor_copy(ob[:], po[:])
                (nc.sync, nc.gpsimd, nc.scalar)[k % 3].dma_start(
                    out_d.ap()[tp][m][:, qg * QN:(qg + 1) * QN], ob[:])

            for t in range(HPC):
                kt = kvp.tile([128, CACHE], bf, tag="kt", name="ktile")
                vt = kvp.tile([128, NKC, HD], bf, tag="vt", name="vtile")
                if t == 0:
                    # criticality-ordered, fine-grained: small first-needed
                    # pieces land fast under bandwidth sharing
                    nc.sync.dma_start(vt[:, 0:2], vt_d.ap()[t][:, 0:2])
                    nc.sync.dma_start(kt[:, 0:585], kt_d.ap()[t][:, 0:585])
                    nc.scalar.dma_start(vt[:, 2:6], vt_d.ap()[t][:, 2:6])
                    nc.sync.dma_start(kt[:, 585:1755],
                                      kt_d.ap()[t][:, 585:1755])
                    nc.scalar.dma_start(vt[:, 6:14], vt_d.ap()[t][:, 6:14])
                    nc.sync.dma_start(kt[:, 1755:3315],
                                      kt_d.ap()[t][:, 1755:3315])
                    nc.scalar.dma_start(vt[:, 14:26], vt_d.ap()[t][:, 14:26])
                    nc.sync.dma_start(kt[:, 3315:CACHE],
                                      kt_d.ap()[t][:, 3315:CACHE])
                    nc.scalar.dma_start(vt[:, 26:37], vt_d.ap()[t][:, 26:37])
                else:
                    nc.sync.dma_start(kt[:], kt_d.ap()[t])
                    nc.gpsimd.dma_start(vt[:], vt_d.ap()[t])

                for qg in range(2):
                    q0 = qg * QN
                    opsum = opsp.tile([128, QN], f32, tag="opsum",
                                      name="opsum")
                    dacc4 = accp.tile([128, 4, QN], bf, tag="dacc",
                                      name="dacc")
                    lps = {}

                    def emit_qk(pj):
                        lp = lpp.tile([128, 2, 512], f32, tag="lp",
                                      name="lp")
                        lps[pj] = lp
                        if pj < NPAIR - 1:
                            for i in range(2):
                                j = 2 * pj + i
                                nc.tensor.matmul(
                                    lp[:, i, 0:QN],
                                    kt[:, j * 128:(j + 1) * 128],
                                    qt[:, t, q0:q0 + QN],
                                    start=True, stop=True)
                        else:
                            j = 2 * pj
                            nc.tensor.matmul(
                                lp[:TAIL, 0, 0:QN],
                                kt[:, j * 128:j * 128 + TAIL],
                                qt[:, t, q0:q0 + QN],
                                start=True, stop=True)

                    for pj in range(min(DEPTH + 1, NPAIR)):
                        emit_qk(pj)

                    for du in range(NQUAD):
                        ptq = pp.tile([128, 4, QN], bf, tag="pt", name="pt")
                        for half in range(2):
                            pj = 2 * du + half
                            if pj >= NPAIR:
                                continue
                            lp = lps.pop(pj)
                            last = pj == NPAIR - 1
                            sl = slice(2 * half, 2 * half + 2)
                            if not last:
                                if pj in SCH_PAIRS:
                                    nc.vector.tensor_scalar(
                                        out=ptq[:, sl, :].bitcast(u16),
                                        in0=lp[:, :, 0:QN],
                                        scalar1=float(SCH_A16),
                                        scalar2=float(SCH_B16),
                                        op0=AluOpType.mult,
                                        op1=AluOpType.add)
                                else:
                                    nc.scalar.activation(
                                        out=ptq[:, sl, :],
                                        in_=lp[:, :, 0:QN],
                                        func=AF.Exp, scale=ATT_SCALE,
                                        bias=ebias[:])
                            else:
                                nc.scalar.activation(
                                    out=ptq[:TAIL, 2 * half, :],
                                    in_=lp[:TAIL, 0, 0:QN],
                                    func=AF.Exp, scale=ATT_SCALE,
                                    bias=ebias[:TAIL])
                            if not last:
                                for i in range(2):
                                    j = 2 * pj + i
                                    nc.tensor.matmul(
                                        opsum[:], vt[:, j, :],
                                        ptq[:, 2 * half + i, :],
                                        start=(j == 0), stop=False)
                            else:
                                j = 2 * pj
                                nc.tensor.matmul(
                                    opsum[:], vt[0:TAIL, j, :],
                                    ptq[:TAIL, 2 * half, :],
                                    start=False, stop=True)
                            if pj + DEPTH + 1 < NPAIR:
                                emit_qk(pj + DEPTH + 1)
                        # one denominator fold per quad (FD=1560, bf16 2x)
                        if du == 0:
                            nc.vector.tensor_copy(dacc4[:], ptq[:])
                        elif 2 * du + 1 < NPAIR:
                            nc.vector.tensor_tensor(
                                dacc4[:], dacc4[:], ptq[:], AluOpType.add)
                        else:
                            nc.vector.tensor_tensor(
                                dacc4[:TAIL, 0, :], dacc4[:TAIL, 0, :],
                                ptq[:TAIL, 0, :], AluOpType.add)
                        # deferred boundary work from the previous block,
                        # emitted once the PE is warm again
                        if du == 1 and pending_dred:
                            dp, tp, qp = pending_dred.pop(0)
                            dred = lpp.tile([128, 2, 512], f32, tag="lp",
                                            name="dred")
                            nc.tensor.matmul(dred[0:1, 0, 0:QN],
                                             ones_bf[:], dp[:, 0, :],
                                             start=True, stop=True)
                            dsb = outsp.tile([1, QN], f32, tag="dsb",
                                             name="dsb")
                            nc.vector.tensor_copy(dsb[:],
                                                  dred[0:1, 0, 0:QN])
                            nc.gpsimd.dma_start(ds_d.ap()[tp][qp], dsb[:])
                        # interleave ready o-projection halves
                        if du >= 1:
                            emit_oproj_half()
                            if du >= 3:
                                emit_oproj_half()

                    # combine fold slots (VectorE); the partition-reduce
                    # matmul is deferred into the next block
                    for sl2 in (1, 2, 3):
                        nc.vector.tensor_tensor(
                            dacc4[:, 0, :], dacc4[:, 0, :],
                            dacc4[:, sl2, :], AluOpType.add)
                    pending_dred.append((dacc4, t, qg))
                    # stage O^T bf16 for the o-projection
                    nc.vector.tensor_copy(o3[:, t, q0:q0 + QN], opsum[:])
                    oproj_ready.extend((t, m, qg) for m in range(12))

            # flush remaining denominators and the last head's o-projection
            for dp, tp, qp in pending_dred:
                dred = lpp.tile([128, 2, 512], f32, tag="lp", name="dred")
                nc.tensor.matmul(dred[0:1, 0, 0:QN], ones_bf[:],
                                 dp[:, 0, :], start=True, stop=True)
                dsb = outsp.tile([1, QN], f32, tag="dsb", name="dsb")
                nc.vector.tensor_copy(dsb[:], dred[0:1, 0, 0:QN])
                nc.gpsimd.dma_start(ds_d.ap()[tp][qp], dsb[:])
            k = 0
            while oproj_ready:
                emit_oproj_half(tag=("po", "opsum")[k % 2])
                k += 1

    nc.finalize()
    return nc


# ---------------------------------------------------------------------------
def _cache_plan(current_start, global_end_index, local_end_index, s, kv_size,
                frame_seqlen):
    """Numpy re-implementation of the reference's cache roll/update/window
    logic, tracking only *indices*: returns (old_cache_rows, new_rows) such
    that the attended key set == cache[old_cache_rows] ++ new[new_rows]."""
    current_end = current_start + s
    sink_tokens = SINK_SIZE * frame_seqlen

    kind = np.zeros(kv_size, dtype=np.int64)
    idx = np.arange(kv_size, dtype=np.int64)

    if (LOCAL_ATTN_SIZE != -1 and current_end > global_end_index
            and s + local_end_index > kv_size):
        num_evicted = s + local_end_index - kv_size
        num_rolled = local_end_index - num_evicted - sink_tokens
        src0 = sink_tokens + num_evicted
        kind[sink_tokens:sink_tokens + num_rolled] = \
            kind[src0:src0 + num_rolled]
        idx[sink_tokens:sink_tokens + num_rolled] = \
            idx[src0:src0 + num_rolled]
        new_local_end = (local_end_index + current_end - global_end_index
                         - num_evicted)
    else:
        new_local_end = local_end_index + current_end - global_end_index
    local_start = new_local_end - s
    is_recompute = (current_end <= global_end_index) and (current_start > 0)
    write_start = max(local_start, sink_tokens) if is_recompute \
        else local_start
    off = max(0, write_start - local_start)
    wl = max(0, new_local_end - write_start)
    if wl > 0:
        kind[write_start:new_local_end] = 1
        idx[write_start:new_local_end] = off + np.arange(wl)

    if sink_tokens > 0:
        budget = MAX_ATTN - sink_tokens
        if budget > 0:
            lo = max(sink_tokens, new_local_end - budget)
            sel = np.concatenate([np.arange(sink_tokens),
                                  np.arange(lo, new_local_end)])
        else:
            sel = np.arange(sink_tokens)
    else:
        ws = max(0, new_local_end - MAX_ATTN)
        sel = np.arange(ws, new_local_end)

    k_kind, k_idx = kind[sel], idx[sel]
    old_rows = k_idx[k_kind == 0]
    new_rows = k_idx[k_kind == 1]
    return old_rows, new_rows


def _rope_tables(freqs_real, freqs_imag, f, h, w, start_frame):
    """(S, HD) cos table and sign-folded sin table for one head."""
    c = HD // 2  # 64
    c0 = c - 2 * (c // 3)
    c1 = c // 3
    fr = np.asarray(freqs_real, np.float32)
    fi = np.asarray(freqs_imag, np.float32)
    s = f * h * w
    assert s == S
    fidx = np.arange(s) // (h * w)
    hidx = (np.arange(s) // w) % h
    widx = np.arange(s) % w
    fr_pos = np.concatenate([
        fr[start_frame + fidx][:, :c0],
        fr[hidx][:, c0:c0 + c1],
        fr[widx][:, c0 + c1:c0 + 2 * c1],
    ], axis=1)  # (S, 64)
    fi_pos = np.concatenate([
        fi[start_frame + fidx][:, :c0],
        fi[hidx][:, c0:c0 + c1],
        fi[widx][:, c0 + c1:c0 + 2 * c1],
    ], axis=1)
    C1 = np.repeat(fr_pos, 2, axis=1)              # (S, 128) cos
    Sg = np.empty((s, HD), np.float32)
    Sg[:, 0::2] = -fi_pos                          # y_even = xe*c - xo*si
    Sg[:, 1::2] = fi_pos                           # y_odd  = xo*c + xe*si
    return C1, Sg


def _rope_apply(x, C, Sx, g):
    """x: (S, DIM) float32; returns rope(x*g) per head with g folded."""
    gx = x * np.asarray(g, np.float32)[None, :]
    xs = gx.reshape(S, NH, HD // 2, 2)
    sw = xs[..., ::-1].reshape(S, NH, HD)          # swapped pairs
    xr = gx.reshape(S, NH, HD)
    return (xr * C[:, None, :] + sw * Sx[:, None, :]).reshape(S, DIM)


# ---------------------------------------------------------------------------
def kernel(x, cache_k, cache_v, freqs_real, freqs_imag,
           wq, bq, wk, bk, wv, bv, wo, bo, gq, gk,
           f_frames, height, width, current_start, global_end_index,
           local_end_index):
    global LAST_RUNS
    LAST_RUNS = []

    x = np.asarray(x, np.float32)
    cache_k = np.asarray(cache_k, np.float32)
    cache_v = np.asarray(cache_v, np.float32)
    wq = np.asarray(wq, np.float32)
    wk = np.asarray(wk, np.float32)
    wv = np.asarray(wv, np.float32)
    wo = np.asarray(wo, np.float32)
    bo = np.asarray(bo, np.float32)
    f = int(f_frames)
    h = int(height)
    w = int(width)
    current_start = int(current_start)
    global_end_index = int(global_end_index)
    local_end_index = int(local_end_index)

    assert x.shape == (1, S, DIM)
    for b in (bq, bk, bv):
        assert not np.any(np.asarray(b)), "nonzero qkv bias unsupported"

    frame_seqlen = h * w
    start_frame = current_start // frame_seqlen

    # ---- launch 1: q/k/v projections (column-sharded, bf16) ----
    xT = np.ascontiguousarray(x[0].T)                       # (1536, 1560)
    xtp = np.ascontiguousarray(
        xT.reshape(12, 128, S).transpose(1, 0, 2)).astype(BF16)

    nc1 = _CACHED.get("l1")
    if nc1 is None:
        nc1 = _CACHED["l1"] = _build_launch1()

    in_maps1 = []
    for c in range(NCORES):
        cs = slice(c * CPC, (c + 1) * CPC)
        W_slice = np.concatenate([wq[:, cs], wk[:, cs], wv[:, cs]], axis=1)
        wp = np.ascontiguousarray(
            W_slice.reshape(12, 128, L1_COLS).transpose(1, 0, 2)).astype(BF16)
        in_maps1.append({"xt": xtp, "wp": wp})
    res1 = bass_utils.run_bass_kernel_spmd(nc1, in_maps1,
                                           core_ids=list(range(NCORES)))
    LAST_RUNS.append(res1)

    Q = np.empty((S, DIM), np.float32)
    K = np.empty((S, DIM), np.float32)
    V = np.empty((S, DIM), np.float32)
    for c in range(NCORES):
        cs = slice(c * CPC, (c + 1) * CPC)
        blk = res1.results[c]["qkv"].reshape(S, L1_COLS).astype(np.float32)
        Q[:, cs] = blk[:, 0:CPC]
        K[:, cs] = blk[:, CPC:2 * CPC]
        V[:, cs] = blk[:, 2 * CPC:3 * CPC]

    # ---- host glue: rms + rope + cache assembly ----
    rs_q = 1.0 / np.sqrt(np.mean(Q * Q, axis=1, keepdims=True) + EPS)
    rs_k = 1.0 / np.sqrt(np.mean(K * K, axis=1, keepdims=True) + EPS)
    C1, Sg = _rope_tables(freqs_real, freqs_imag, f, h, w, start_frame)
    Qr = _rope_apply(Q, C1, Sg, gq) * rs_q
    Kr = _rope_apply(K, C1, Sg, gk) * rs_k

    old_rows, new_rows = _cache_plan(current_start, global_end_index,
                                     local_end_index, S, cache_k.shape[1],
                                     frame_seqlen)
    n_old = len(old_rows)
    assert n_old + len(new_rows) == CACHE, "unexpected key count"

    K_eff = np.empty((CACHE, DIM), np.float32)
    V_eff = np.empty((CACHE, DIM), np.float32)
    K_eff[:n_old] = cache_k[0, old_rows].reshape(n_old, DIM)
    K_eff[n_old:] = Kr[new_rows]
    V_eff[:n_old] = cache_v[0, old_rows].reshape(n_old, DIM)
    V_eff[n_old:] = V[new_rows]

    Q8 = Qr.astype(BF16)                                    # (S, DIM)
    K8 = K_eff.astype(BF16)
    V8 = V_eff.astype(BF16)

    # launch-2 layouts
    V_pad = np.zeros((NKC * 128, DIM), BF16)
    V_pad[:CACHE] = V8
    # vt[h, p, j, d] = V[j*128 + p, h*128 + d]
    vt_all = np.ascontiguousarray(
        V_pad.reshape(NKC, 128, NH, HD).transpose(2, 1, 0, 3))
    # kt[h, d, key]
    kt_all = np.ascontiguousarray(
        K8.T.reshape(NH, HD, CACHE))
    # qt[h, d, row]
    qt_all = np.ascontiguousarray(Q8.T.reshape(NH, HD, S))
    wo_bf = wo.astype(BF16)

    nc2 = _CACHED.get("l2")
    if nc2 is None:
        nc2 = _CACHED["l2"] = _build_launch2()

    in_maps2 = []
    for c in range(NCORES):
        qg, hg = divmod(c, 4)
        hs = slice(hg * HPC, (hg + 1) * HPC)
        w2 = np.ascontiguousarray(
            wo_bf[hg * HPC * 128:(hg + 1) * HPC * 128].reshape(
                HPC, 128, 12, 128).transpose(1, 0, 2, 3))
        in_maps2.append({
            "qt": np.ascontiguousarray(
                qt_all[hs, :, qg * QB:(qg + 1) * QB].transpose(1, 0, 2)),
            "kt": np.ascontiguousarray(kt_all[hs]),
            "vt": np.ascontiguousarray(vt_all[hs]),
            "w2": w2,
        })
    res2 = bass_utils.run_bass_kernel_spmd(nc2, in_maps2,
                                           core_ids=list(range(NCORES)))
    LAST_RUNS.append(res2)

    # ---- host: normalize by denominators, reduce heads, add bo ----
    out = np.zeros((S, DIM), np.float32)
    for c in range(NCORES):
        qg, hg = divmod(c, 4)
        o_part = res2.results[c]["outp"].astype(np.float32)  # [3,12,128,QB]
        dsum = res2.results[c]["dsum"].reshape(HPC, QB)      # [3, QB]
        rows = slice(qg * QB, (qg + 1) * QB)
        acc = (o_part / dsum[:, None, None, :]).sum(axis=0)  # [12,128,QB]
        out[rows] += acc.reshape(DIM, QB).T
    out += bo[None, :]
    return out.reshape(1, S, DIM)
